# revision 6
# baseline (speedup 1.0000x reference)
"""DSSIM loss kernel for Trainium2, 8 NeuronCores — v2.

Strategy vs v1 baseline (705us), per trace analysis:
  - Host fields {x+y, x-y, 2xy, x^2+y^2}: the sigma algebra becomes linear
    in the blurred fields, so the pointwise stage is 7 TT + 2 TS + 1 accum
    on DVE/GPSIMD (v1 used scalar_tensor_tensor at ~4us each).
  - ACT uses only Copy/Square/Reciprocal — all live in the single
    `reciprocal_and_small` activation-table set: removes 31 ACT_TABLE_LOADs
    (47us + serialization) and does the division in one pass (v1: Ln+Exp).
    Reciprocal is emitted raw (bass guards it for accuracy; DSSIM slack is
    huge: ssim~0.007, so 1% recip error moves the loss ~7e-5).
  - All pointwise math scaled by 8 (folded into free scale slots) so 1/den
    stays < ~250 and fits fp16 comfortably.
  - PSUM: both passes tiled in h'-halves (135) with two fields packed per
    2KB bank; pv and p2 pools double-buffered (4+4 banks) so the tensor
    engine never waits on evacuations (v1: 529us MATMUL semaphore wait,
    single-buffered PSUM).
  - pass1 evac: ACT Copy, one instr per packed field-pair.
  - pass2 "evac" fused with math: Square(psum) on ACT for the mean fields,
    one 2-op tensor_scalar from PSUM on DVE for the second-moment fields.
  - Final sum via tensor_scalar accum_out into per-group acc columns.
"""

import sys

sys.path.insert(0, "/opt/trn_rl_repo")

import numpy as np

import concourse.bass as bass
import concourse.bacc as bacc
import concourse.tile as tile
from concourse import mybir
from concourse.bass_utils import run_bass_kernel_spmd

# ---------------- problem geometry (hardcoded) ----------------
H, W, C = 2160, 3840, 3
WC = W * C  # 11520
NCORES = 8
HOUT = H // NCORES  # 270
HSLAB = HOUT + 10  # 280 input rows per core (5 halo each side)
PADL = 15  # left zero pad in wc (= 5 pixels * 3 channels)
TW = 98  # output tile width in wc' (128 - 2*15)
NT = (WC + TW - 1) // TW  # 118 tiles; last tile has 54 valid columns
WPAD = TW * (NT - 1) + 128  # 11594
WPAD = ((WPAD + 7) // 8) * 8  # 11600
TGROUP = 8
NB = (NT + TGROUP - 1) // TGROUP  # 15 groups
NV = HOUT  # 270 (h' extent)
HNV = NV // 2  # 135 (h' half)
WIN, SIGMA = 11, 1.5
C1, C2 = 0.01**2, 0.03**2
SC = 8.0  # range scale folded into num and den (cancels in the ratio)

# input row-blocks (baseline 3-block layout):
IBLOCKS = [(0, 128), (118, 128), (236, 44)]
# uneven h'-halves split at the natural block boundary, so half 0 needs a
# single K=128 matmul per field: (hoff, hwidth) and per-half matmul lists
# of (input_block_idx, K, out_col, M)
HALVES = [(0, 118), (118, 152)]
HBLOCKS = {
    0: [(0, 128, 0, 118)],
    1: [(1, 128, 0, 118), (2, 44, 118, 34)],
}

F16 = mybir.dt.float16
F32 = mybir.dt.float32

FIELDS = ("p", "m", "q", "s")  # x+y, x-y, 2xy, x^2+y^2


def _gauss():
    co = np.arange(WIN, dtype=np.float64) - (WIN // 2)
    g = np.exp(-(co**2) / (2.0 * SIGMA**2))
    return (g / g.sum()).astype(np.float32)


def _band_gain():
    g16 = _gauss().astype(np.float16).astype(np.float64)
    return float(g16.sum())


def _bands():
    g = _gauss()
    bv = np.zeros((128, 118), dtype=np.float32)
    for k in range(128):
        for m in range(118):
            t = k - m
            if 0 <= t <= 10:
                bv[k, m] = g[t]
    bh = np.zeros((128, TW), dtype=np.float32)
    for k in range(128):
        for m in range(TW):
            d3 = k - m - 15
            if d3 % 3 == 0 and -15 <= d3 <= 15:
                bh[k, m] = g[d3 // 3 + 5]
    return bv.astype(np.float16), bh.astype(np.float16)


def _act_raw(eng, out, in_, func, scale=1.0):
    """Emit InstActivation directly (bypasses the bass Reciprocal guard)."""
    ins = [
        eng.lower_ap(in_),
        mybir.ImmediateValue(dtype=mybir.dt.float32, value=0.0),  # bias
        mybir.ImmediateValue(dtype=mybir.dt.float32, value=scale),  # scale
        mybir.ImmediateValue(dtype=mybir.dt.float32, value=0.0),  # alpha
    ]
    return eng.add_instruction(
        mybir.InstActivation(
            name=eng.bass.get_next_instruction_name(),
            func=func,
            ins=ins,
            outs=[eng.lower_ap(out)],
        )
    )


def _pair_view(ap2d, width):
    """[P, >=2*width] -> [P, 2, width] (field-pair view of the first 2w)."""
    return ap2d[:, 0 : 2 * width].rearrange("p (f v) -> p f v", v=width)


def _gpair(t, gw, col, width):
    """group-buffer pair view: [P, 2*gw] -> [P, 2, width] at (col, gw+col)."""
    return t[:, :].rearrange("p (f v) -> p f v", v=gw)[:, :, col : col + width]


def _body(tc, ins, outs):
    nc = tc.nc
    fin = {f: ins[f"f{f}"] for f in FIELDS}
    bv_d, bh_d = ins["bv"], ins["bh"]
    acc_out = outs["acc"]
    ADD, SUB, MUL = (
        mybir.AluOpType.add,
        mybir.AluOpType.subtract,
        mybir.AluOpType.mult,
    )
    ACTF = mybir.ActivationFunctionType
    s = _band_gain()
    SCL = 1.0 / (s * s)  # cancel per-pass f16 band gain at pass1 evac
    KSQ = float(np.sqrt(SC / 2.0))  # Square scale: (KSQ*m)^2 = SC*m^2/2
    C1S = SC * C1
    C2S = SC * C2

    consts = tc.alloc_tile_pool(name="consts", bufs=1)
    inp = tc.alloc_tile_pool(name="inp", bufs=3)
    tvp = tc.alloc_tile_pool(name="tv", bufs=3)
    pvp = tc.alloc_tile_pool(name="pv", bufs=2, space="PSUM")
    p2p = tc.alloc_tile_pool(name="p2", bufs=1, space="PSUM")
    sbp = tc.alloc_tile_pool(name="sb", bufs=2)
    pwp = tc.alloc_tile_pool(name="pw", bufs=2)

    bv_s = consts.tile([128, 118], F16)
    nc.sync.dma_start(out=bv_s, in_=bv_d[:, :])
    bh_s = consts.tile([128, TW], F16)
    nc.sync.dma_start(out=bh_s, in_=bh_d[:, :])
    acc = consts.tile([128, NB], F32)
    nc.vector.memset(acc, 0.0)
    msk_s = consts.tile([128, 2], F32)
    nc.sync.dma_start(out=msk_s, in_=ins["msk"][:, :])

    WGMAX = TW * TGROUP + 30  # 814
    GW = NV * TGROUP  # 2160 columns per field in group buffers

    for grp in range(NB):
        t0 = grp * TGROUP
        nt = min(TGROUP, NT - t0)
        wg0 = TW * t0
        WG = TW * nt + 30
        W2 = NV * nt

        # ---- load input strips: 4 fields x 4 row-blocks ----
        itiles = {}
        for fname in FIELDS:
            for bi, (rs, K) in enumerate(IBLOCKS):
                t = inp.tile(
                    [128, WGMAX], F16, tag=f"in_{fname}_{bi}",
                    name=f"in_{fname}_{bi}_{grp}",
                )
                nc.sync.dma_start(
                    out=t[:K, :WG], in_=fin[fname][rs : rs + K, wg0 : wg0 + WG]
                )
                itiles[(fname, bi)] = t

        # group buffers: field A in cols [0,GW), field B in cols [GW,2GW)
        sbAB = sbp.tile([TW, 2 * GW], F16, tag="sbAB", name=f"sbAB_{grp}")  # A'|B'
        sb34 = sbp.tile([TW, 2 * GW], F16, tag="sb34", name=f"sb34_{grp}")  # S3'|S4'

        for ti in range(nt):
            o = TW * ti
            # tv: pass1 results, [128, 2*NV] per field pair
            tvAB = tvp.tile([128, 2 * NV], F16, tag="tvAB", name=f"tvAB_{grp}_{ti}")
            tvCD = tvp.tile([128, 2 * NV], F16, tag="tvCD", name=f"tvCD_{grp}_{ti}")
            for h, (hoff, hw) in enumerate(HALVES):
                # ---- pass1 (vertical blur), h'-half, fields packed in pairs
                pvm = pvp.tile([128, 304], F32, tag="pvm", name=f"pvm_{grp}_{ti}_{h}")
                pvs = pvp.tile([128, 304], F32, tag="pvs", name=f"pvs_{grp}_{ti}_{h}")
                for pv_t, fpair in ((pvm, ("p", "m")), (pvs, ("q", "s"))):
                    for fi, fname in enumerate(fpair):
                        fc = fi * hw
                        for bi, K, hp, M in HBLOCKS[h]:
                            nc.tensor.matmul(
                                pv_t[:, fc + hp : fc + hp + M],
                                itiles[(fname, bi)][:K, o : o + 128],
                                bv_s[:K, :M],
                                start=True,
                                stop=True,
                            )
                # ---- pass1 evac: one packed-pair instr per engine ----
                nc.scalar.activation(
                    _pair_view(tvAB, NV)[:, :, hoff : hoff + hw],
                    _pair_view(pvm, hw),
                    ACTF.Copy,
                    scale=SCL,
                )
                nc.vector.tensor_scalar(
                    _pair_view(tvCD, NV)[:, :, hoff : hoff + hw],
                    _pair_view(pvs, hw),
                    SCL,
                    None,
                    MUL,
                )
            # ---- pass2 (horizontal blur): full-height, one MM per field
            p2t = {}
            for fi, fname in enumerate(FIELDS):
                pt = p2p.tile([TW, NV], F32, tag=f"p2{fname}", name=f"p2{fname}_{grp}_{ti}")
                tv_t = tvAB if fi < 2 else tvCD
                nc.tensor.matmul(
                    pt,
                    bh_s,
                    tv_t[:, (fi % 2) * NV : (fi % 2) * NV + NV],
                    start=True,
                    stop=True,
                )
                p2t[fname] = pt
            # ---- stage2: fused evac + first pointwise layer ----
            colA = ti * NV
            nc.scalar.activation(
                sbAB[:, colA : colA + NV], p2t["p"], ACTF.Square, scale=KSQ
            )
            nc.scalar.activation(
                sbAB[:, GW + colA : GW + colA + NV], p2t["m"], ACTF.Square, scale=KSQ
            )
            nc.vector.tensor_scalar(
                sb34[:, colA : colA + NV], p2t["q"], SC, C2S, MUL, ADD
            )
            nc.vector.tensor_scalar(
                sb34[:, GW + colA : GW + colA + NV], p2t["s"], SC, C2S, MUL, ADD
            )

        # ---- group pointwise stage on [98, W2] fp16 ----
        Av = sbAB[:, 0:W2]
        Bv = sbAB[:, GW : GW + W2]
        S3 = sb34[:, 0:W2]
        S4 = sb34[:, GW : GW + W2]
        al0 = pwp.tile([TW, GW], F16, tag="al0", name=f"al0_{grp}")
        u0t = pwp.tile([TW, GW], F16, tag="u0t", name=f"u0t_{grp}")
        alt = pwp.tile([TW, GW], F16, tag="alt", name=f"alt_{grp}")
        u1t = pwp.tile([TW, GW], F16, tag="u1t", name=f"u1t_{grp}")
        rnt = pwp.tile([TW, GW], F16, tag="rnt", name=f"rnt_{grp}")
        rdt = pwp.tile([TW, GW], F16, tag="rdt", name=f"rdt_{grp}")
        numt = pwp.tile([TW, GW], F16, tag="numt", name=f"numt_{grp}")
        dent = pwp.tile([TW, GW], F16, tag="dent", name=f"dent_{grp}")
        rect = pwp.tile([TW, GW], F16, tag="rect", name=f"rect_{grp}")

        # al0 = A' - B' = SC * mux*muy              (DVE)
        nc.vector.tensor_tensor(al0[:, :W2], Av, Bv, SUB)
        # u0 = A' + B' = SC/2 * (mux^2 + muy^2)     (GPSIMD)
        nc.vector.tensor_tensor(u0t[:, :W2], Av, Bv, ADD)
        # rn = S3' - 2*al0 ... careful with factors, see note below
        # S3' = SC*(2*blur(xy)) + SC*C2 ; al0 = SC*(2 mux muy)/1 ?
        # al0 = (SC/2)(m1^2 - m2^2)/1 = SC/2 * 4 mux muy = 2*SC*mux*muy
        # => rn = S3' - al0 = SC*(2 sigxy + C2)     (DVE)
        nc.vector.tensor_tensor(rnt[:, :W2], S3, al0[:, :W2], SUB)
        # al = al0 + SC*C1 = SC*(2 mux muy + C1)    (DVE TS)
        nc.vector.tensor_scalar(alt[:, :W2], al0[:, :W2], 1.0, C1S, MUL, ADD)
        # rd = S4' - u0 = SC*(sigxx + sigyy + C2)   (GPSIMD, after u0)
        nc.vector.tensor_tensor(rdt[:, :W2], S4, u0t[:, :W2], SUB)
        # u1 = u0 + SC*C1 = SC*(mux^2+muy^2 + C1)   (DVE TS)
        nc.vector.tensor_scalar(u1t[:, :W2], u0t[:, :W2], 1.0, C1S, MUL, ADD)
        # num = al * rn                              (DVE)
        nc.vector.tensor_tensor(numt[:, :W2], alt[:, :W2], rnt[:, :W2], MUL)
        # den = u1 * rd                              (DVE)
        nc.vector.tensor_tensor(dent[:, :W2], u1t[:, :W2], rdt[:, :W2], MUL)
        if t0 + nt == NT:
            # last output tile (t=117): only 54 valid wc' partitions.
            # num <- num*m ; den <- den*m + (1-m)  (m: 1 valid / 0 invalid)
            iv0 = NV * (NT - 1 - t0)
            nc.vector.tensor_scalar(
                numt[:TW, iv0 : iv0 + NV], numt[:TW, iv0 : iv0 + NV],
                msk_s[:TW, 0:1], 0.0, MUL, ADD,
            )
            nc.vector.tensor_scalar(
                dent[:TW, iv0 : iv0 + NV], dent[:TW, iv0 : iv0 + NV],
                msk_s[:TW, 0:1], msk_s[:TW, 1:2], MUL, ADD,
            )
        # rec = 1/den                                (ACT Reciprocal, raw)
        _act_raw(nc.scalar, rect[:, :W2], dent[:, :W2], ACTF.Reciprocal)
        # ssim = num * rec; acc[:, grp] += sum      (DVE TT + TS accum)
        nc.vector.tensor_tensor(numt[:, :W2], numt[:, :W2], rect[:, :W2], MUL)
        nc.scalar.activation(
            al0[:, :W2], numt[:, :W2], ACTF.Copy,
            accum_out=acc[:TW, grp : grp + 1],
        )

    nc.sync.dma_start(out=acc_out[:, :], in_=acc)

    for p in (pwp, sbp, p2p, pvp, tvp, inp, consts):
        p.release()


_CACHE = {}


def _get_compiled():
    if "nc" in _CACHE:
        return _CACHE["nc"], _CACHE["aps"]
    nc = bacc.Bacc("TRN2", target_bir_lowering=False, debug=False, num_devices=NCORES)
    ins = {}
    for f in FIELDS:
        ins[f"f{f}"] = nc.dram_tensor(
            f"f{f}", [HSLAB, WPAD], F16, kind="ExternalInput"
        ).ap()
    ins["bv"] = nc.dram_tensor("bv", [128, 118], F16, kind="ExternalInput").ap()
    ins["bh"] = nc.dram_tensor("bh", [128, TW], F16, kind="ExternalInput").ap()
    ins["msk"] = nc.dram_tensor("msk", [128, 2], F32, kind="ExternalInput").ap()
    outs = {"acc": nc.dram_tensor("acc", [128, NB], F32, kind="ExternalOutput").ap()}
    with tile.TileContext(nc) as tc:
        _body(tc, ins, outs)
    nc.compile()
    _CACHE["nc"] = nc
    _CACHE["aps"] = (ins, outs)
    return nc, (ins, outs)


LAST_RES = None


def kernel(X, Y, _trace=False, _trace_kwargs=None):
    global LAST_RES
    X = np.asarray(X, dtype=np.float32).reshape(H, WC)
    Y = np.asarray(Y, dtype=np.float32).reshape(H, WC)

    bv, bh = _bands()
    nvalid = WC - TW * (NT - 1)  # 54
    msk = np.zeros((128, 2), dtype=np.float32)
    msk[:nvalid, 0] = 1.0
    msk[:, 1] = 1.0 - msk[:, 0]
    fields = {
        "fp": X + Y,
        "fm": X - Y,
        "fq": 2.0 * (X * Y),
        "fs": X * X + Y * Y,
    }
    # pad rows (5 top/bottom) and wc (15 left, to WPAD right), cast fp16
    padded = {}
    for k, a in fields.items():
        p = np.zeros((H + 10, WPAD), dtype=np.float16)
        p[5 : 5 + H, PADL : PADL + WC] = a.astype(np.float16)
        padded[k] = p

    in_maps = []
    for c in range(NCORES):
        m = {
            k: np.ascontiguousarray(p[HOUT * c : HOUT * c + HSLAB])
            for k, p in padded.items()
        }
        m["bv"] = bv
        m["bh"] = bh
        m["msk"] = msk
        in_maps.append(m)

    nc, _ = _get_compiled()
    res = run_bass_kernel_spmd(
        nc, in_maps, core_ids=list(range(NCORES)),
        trace=_trace, **(_trace_kwargs or {}),
    )
    LAST_RES = res
    total = 0.0
    for r in res.results:
        total += float(np.asarray(r["acc"])[:TW, :].astype(np.float64).sum())
    # acc holds SC*num/(SC*den) = ssim, summed over all pixels
    loss = 1.0 - total / (H * W * C)
    return np.float32(loss)
